# revision 3
# baseline (speedup 1.0000x reference)
"""Bidirectional Mamba on 8 TRN2 NeuronCores — v2.

Core c handles (batch b = c%4, direction d = c//4); per-layer bidirectional
merge is a pair AllReduce. Feature-major layout [channels-on-partitions,
tokens-on-free] throughout.

v2 changes vs baseline:
 - all activations SBUF-resident (u/z/y/g never round-trip DRAM)
 - conv NOT folded into in_proj: plain 2*DI in_proj (288 N=512 matmuls),
   depthwise conv as 4 shifted STT ops per fold on DVE
 - dt path sigmoid-free: q=exp(dt+dtb) [ACT], r=q+1 [DVE TS], delta=ln(r)
   [ACT], dA_i=exp(-a_i*delta) [ACT] -- only exp/ln (one ACT table set)
   plus silu; no sigmoid/ln set thrash
 - B broadcast tiles built once per layer ([P,DS,L] resident); C broadcast
   per (chunk,state) from a small pool
 - dt_proj per chunk (K=48 matmuls), logp/w transient per chunk
"""
import os
import numpy as np
import ml_dtypes
from contextlib import ExitStack

import concourse.bacc as bacc
import concourse.tile as tile
import concourse.mybir as mybir
from concourse.bass_utils import run_bass_kernel_spmd

F32 = mybir.dt.float32
BF16 = mybir.dt.bfloat16
MUL = mybir.AluOpType.mult
ADD = mybir.AluOpType.add
SUB = mybir.AluOpType.subtract
AF = mybir.ActivationFunctionType

P = 128
B, L, D, DI, DS, DR, DC, NL = 4, 1024, 768, 1536, 16, 48, 4, 4
EPS = 1e-5
KF = D // P        # 6  k-folds of d_model
MF = DI // P       # 12 folds of d_inner
NT = L // 512      # 2  N-tiles per token row
CH = 2             # folds per scan chunk
NCH = MF // CH     # 6 chunks
MARG = DC - 1      # 3 left-pad columns for causal conv

_CACHE = {}


def _build():
    KNL = int(os.environ.get("K_NL", NL))
    nc = bacc.Bacc("TRN2", target_bir_lowering=False)

    # ---------------- dram I/O ----------------
    x0T_d = nc.dram_tensor("x0T", [D, L], F32, kind="ExternalInput")
    lnw_d = nc.dram_tensor("lnw", [NL, P, KF], F32, kind="ExternalInput")
    lnb_d = nc.dram_tensor("lnb", [NL, P, KF], F32, kind="ExternalInput")
    inwT_d = nc.dram_tensor("inwT", [NL, D, 2 * DI], BF16, kind="ExternalInput")
    cb_d = nc.dram_tensor("cb", [NL, P, MF], F32, kind="ExternalInput")
    xpwT_d = nc.dram_tensor("xpwT", [NL, DI, 96], BF16, kind="ExternalInput")
    dtwT_d = nc.dram_tensor("dtwT", [NL, DR, DI], BF16, kind="ExternalInput")
    dtbn_d = nc.dram_tensor("dtbn", [NL, P, MF], F32, kind="ExternalInput")
    napos_d = nc.dram_tensor("napos", [NL, P, DS], F32, kind="ExternalInput")
    dsk_d = nc.dram_tensor("dsk", [NL, P, MF], F32, kind="ExternalInput")
    owT_d = nc.dram_tensor("owT", [NL, DI, D], BF16, kind="ExternalInput")
    fw_d = nc.dram_tensor("fw", [P, KF], F32, kind="ExternalInput")
    fb_d = nc.dram_tensor("fb", [P, KF], F32, kind="ExternalInput")
    sel_d = nc.dram_tensor("sel", [P, 2], F32, kind="ExternalInput")
    cwf_d = nc.dram_tensor("cwf", [NL, P, MF, DC], F32, kind="ExternalInput")
    o_d = nc.dram_tensor("o_fm", [D, L], F32, kind="ExternalOutput")

    bc_dram = nc.dram_tensor("bc_sp", [2 * DS, L], BF16)
    cc_in = [nc.dram_tensor(f"cc_in{j}", [D, L], F32) for j in range(NL)]
    cc_out = [nc.dram_tensor(f"cc_out{j}", [D, L], F32) for j in range(NL)]

    with tile.TileContext(nc) as tc, ExitStack() as ctx:
        pers = ctx.enter_context(tc.tile_pool(name="pers", bufs=1))
        vpool = ctx.enter_context(tc.tile_pool(name="vpool", bufs=2))
        big = ctx.enter_context(tc.tile_pool(name="big", bufs=1))
        act = ctx.enter_context(tc.tile_pool(name="act", bufs=1))
        stg = ctx.enter_context(tc.tile_pool(name="stg", bufs=3))
        stg32 = ctx.enter_context(tc.tile_pool(name="stg32", bufs=3))
        wbig = ctx.enter_context(tc.tile_pool(name="wbig", bufs=2))
        prm = ctx.enter_context(tc.tile_pool(name="prm", bufs=1))
        chkp = ctx.enter_context(tc.tile_pool(name="chkp", bufs=2))
        chk1 = ctx.enter_context(tc.tile_pool(name="chk1", bufs=1))
        dApool = ctx.enter_context(tc.tile_pool(name="dApool", bufs=2))
        dBpool = ctx.enter_context(tc.tile_pool(name="dBpool", bufs=2))
        spool = ctx.enter_context(tc.tile_pool(name="spool", bufs=2))
        accp = ctx.enter_context(tc.tile_pool(name="accp", bufs=2))
        cbcp = ctx.enter_context(tc.tile_pool(name="cbcp", bufs=3))
        mm = ctx.enter_context(tc.tile_pool(name="mm", bufs=8, space="PSUM"))

        # persistent across layers
        h32 = pers.tile([P, KF, L], F32, name="h32")
        res16 = vpool.tile([P, KF, L], BF16, name="res16", tag="resp")
        ones16 = pers.tile([P, 1], BF16, name="ones16")
        onef = pers.tile([P, 1], F32, name="onef")
        sel_sb = pers.tile([P, 2], F32, name="sel_sb")
        nc.vector.memset(res16[:], 0.0)
        nc.vector.memset(ones16[:], 1.0)
        nc.vector.memset(onef[:], 1.0)
        nc.sync.dma_start(sel_sb[:], sel_d[:])
        nc.sync.dma_start(h32[:], x0T_d[:].rearrange("(f p) l -> p f l", p=P))

        def ln_feature_major(vin, wcol, bcol, hn_out, out_off):
            """LN over the 768 partition-channels of vin [P,KF,L] (bf16)."""
            ps_s = [mm.tile([P, 512], F32, name="lnps", tag="ps") for _ in range(2 * NT)]
            for f in range(KF):
                sq = stg.tile([P, L], BF16, name="stg_a", tag="st16")
                nc.scalar.activation(sq[:], vin[:, f, :], AF.Square)
                for n in range(NT):
                    nc.tensor.matmul(
                        ps_s[n][0:1, :], ones16[:], vin[:, f, n * 512:(n + 1) * 512],
                        start=(f == 0), stop=(f == KF - 1))
                    nc.tensor.matmul(
                        ps_s[NT + n][0:1, :], ones16[:], sq[:, n * 512:(n + 1) * 512],
                        start=(f == 0), stop=(f == KF - 1))
            mu_r = stg32.tile([1, L], F32, name="mu_r", tag="st32")
            for n in range(NT):
                nc.vector.tensor_scalar(
                    out=mu_r[:, n * 512:(n + 1) * 512], in0=ps_s[n][0:1, :],
                    scalar1=1.0 / D, scalar2=None, op0=MUL)
            mu2_r = stg32.tile([1, L], F32, name="mu2_r", tag="st32")
            nc.vector.tensor_tensor(out=mu2_r[:], in0=mu_r[:], in1=mu_r[:], op=MUL)
            var_r = stg32.tile([1, L], F32, name="var_r", tag="st32")
            for n in range(NT):
                nc.vector.scalar_tensor_tensor(
                    out=var_r[:, n * 512:(n + 1) * 512], in0=ps_s[NT + n][0:1, :],
                    scalar=1.0 / D, in1=mu2_r[:, n * 512:(n + 1) * 512],
                    op0=MUL, op1=SUB)
            eps_r = stg32.tile([1, L], F32, name="eps_r", tag="st32")
            nc.vector.memset(eps_r[:], EPS)
            sd_r = stg32.tile([1, L], F32, name="sd_r", tag="st32")
            nc.scalar.activation(sd_r[:], var_r[:], AF.Sqrt, bias=eps_r[:, 0:1])
            rstd_r = stg32.tile([1, L], F32, name="rstd_r", tag="st32")
            nc.vector.reciprocal(rstd_r[:], sd_r[:])
            mu16_r = stg.tile([1, L], BF16, name="mu16_r", tag="st16")
            rstd16_r = stg.tile([1, L], BF16, name="rstd16_r", tag="st16")
            nc.vector.tensor_copy(mu16_r[:], mu_r[:])
            nc.vector.tensor_copy(rstd16_r[:], rstd_r[:])
            mu_b = cbcp.tile([P, L], BF16, name="mu_b", tag="nbcb")
            rstd_b = cbcp.tile([P, L], BF16, name="rstd_b", tag="nbcb")
            nc.gpsimd.partition_broadcast(mu_b[:], mu16_r[:])
            nc.gpsimd.partition_broadcast(rstd_b[:], rstd16_r[:])
            for f in range(KF):
                st1 = stg.tile([P, L], BF16, name="stg_b", tag="st16")
                nc.vector.tensor_tensor(out=st1[:], in0=vin[:, f, :], in1=mu_b[:], op=SUB)
                st2 = stg.tile([P, L], BF16, name="stg_c", tag="st16")
                nc.vector.tensor_tensor(out=st2[:], in0=st1[:], in1=rstd_b[:], op=MUL)
                nc.vector.scalar_tensor_tensor(
                    out=hn_out[:, f, out_off:out_off + L], in0=st2[:],
                    scalar=wcol[:, f:f + 1],
                    in1=bcol[:, f:f + 1].to_broadcast([P, L]),
                    op0=MUL, op1=ADD)

        for j in range(KNL):
            # ---- per-layer params ----
            lnw = prm.tile([P, KF], F32, name="lnw")
            cwf = prm.tile([P, MF, DC], F32, name="cwf")
            lnb = prm.tile([P, KF], F32, name="lnb")
            cbt = prm.tile([P, MF], F32, name="cbt")
            dtbn = prm.tile([P, MF], F32, name="dtbn")
            napos = prm.tile([P, DS], F32, name="napos")
            dsk = prm.tile([P, MF], F32, name="dsk")
            nc.sync.dma_start(lnw[:], lnw_d[j])
            nc.sync.dma_start(cwf[:], cwf_d[j])
            nc.sync.dma_start(lnb[:], lnb_d[j])
            nc.sync.dma_start(cbt[:], cb_d[j])
            nc.sync.dma_start(dtbn[:], dtbn_d[j])
            nc.sync.dma_start(napos[:], napos_d[j])
            nc.sync.dma_start(dsk[:], dsk_d[j])

            # ---- v = sel0*h + sel1*flip(h) + res ; res' = h + flip(h) + 2res
            v16 = big.tile([P, KF, L], BF16, name="v16", tag="bigC")
            res_new = vpool.tile([P, KF, L], BF16, name="res_new", tag="resp")
            for f in range(KF):
                va = stg.tile([P, L], BF16, name="stg_va", tag="st16")
                nc.vector.scalar_tensor_tensor(
                    out=va[:], in0=h32[:, f, :], scalar=sel_sb[:, 0:1],
                    in1=res16[:, f, :], op0=MUL, op1=ADD)
                nc.vector.scalar_tensor_tensor(
                    out=v16[:, f, :], in0=h32[:, f, ::-1],
                    scalar=sel_sb[:, 1:2], in1=va[:], op0=MUL, op1=ADD)
                tmp = stg.tile([P, L], BF16, name="stg_tm", tag="st16")
                nc.vector.tensor_tensor(out=tmp[:], in0=h32[:, f, :],
                                        in1=h32[:, f, ::-1], op=ADD)
                nc.vector.scalar_tensor_tensor(
                    out=res_new[:, f, :], in0=res16[:, f, :], scalar=2.0,
                    in1=tmp[:], op0=MUL, op1=ADD)
            res16 = res_new

            # ---- LN -> hn ----
            hn16 = big.tile([P, KF, L], BF16, name="hn16", tag="bigA")
            ln_feature_major(v16, lnw, lnb, hn16, 0)

            # ---- in_proj x-half (plain) -> conv -> silu -> u16 ----
            u16 = act.tile([P, MF, L], BF16, name="u16", tag="u16")
            zs16 = act.tile([P, MF, L], BF16, name="zs16", tag="zs16")
            for m in range(MF):
                ps = [mm.tile([P, 512], F32, name="ps_ip", tag="ps") for _ in range(NT)]
                wk = wbig.tile([P, KF, P], BF16, name="wk_ip", tag="w")
                nc.sync.dma_start(
                    wk[:], inwT_d[j, :, m * P:(m + 1) * P]
                    .rearrange("(f p) c -> p f c", p=P))
                for k in range(KF):
                    for n in range(NT):
                        nc.tensor.matmul(
                            ps[n], wk[:, k, :],
                            hn16[:, k, n * 512:(n + 1) * 512],
                            start=(k == 0), stop=(k == KF - 1))
                # padded pre-conv row
                xpre = stg.tile([P, MARG + L], BF16, name="stg_xp", tag="stpad")
                nc.vector.memset(xpre[:, 0:MARG], 0.0)
                for n in range(NT):
                    nc.scalar.copy(xpre[:, MARG + n * 512:MARG + (n + 1) * 512], ps[n])
                cacc = stg.tile([P, L], BF16, name="stg_ca", tag="st16")
                nc.vector.scalar_tensor_tensor(
                    out=cacc[:], in0=xpre[:, 0:L], scalar=cwf[:, m, 0:1],
                    in1=cbt[:, m:m + 1].to_broadcast([P, L]), op0=MUL, op1=ADD)
                for tap in range(1, DC):
                    cacc2 = stg.tile([P, L], BF16, name="stg_cb", tag="st16")
                    nc.vector.scalar_tensor_tensor(
                        out=cacc2[:], in0=xpre[:, tap:tap + L],
                        scalar=cwf[:, m, tap:tap + 1], in1=cacc[:], op0=MUL, op1=ADD)
                    cacc = cacc2
                nc.scalar.activation(u16[:, m, :], cacc[:], AF.Silu)

            # ---- in_proj z-half -> silu -> zs16 ----
            for m in range(MF):
                ps = [mm.tile([P, 512], F32, name="ps_ip", tag="ps") for _ in range(NT)]
                wz = wbig.tile([P, KF, P], BF16, name="wz_ip", tag="w")
                nc.sync.dma_start(
                    wz[:], inwT_d[j, :, DI + m * P:DI + (m + 1) * P]
                    .rearrange("(f p) c -> p f c", p=P))
                for k in range(KF):
                    for n in range(NT):
                        nc.tensor.matmul(
                            ps[n], wz[:, k, :],
                            hn16[:, k, n * 512:(n + 1) * 512],
                            start=(k == 0), stop=(k == KF - 1))
                for n in range(NT):
                    nc.scalar.activation(zs16[:, m, n * 512:(n + 1) * 512],
                                         ps[n], AF.Silu)

            # ---- x_proj ----
            ps_xd = [mm.tile([P, 512], F32, name="ps_xd", tag="ps") for _ in range(NT)]
            for k in range(MF):
                xw = wbig.tile([P, 96], BF16, name="xw_xp", tag="w")
                nc.sync.dma_start(xw[:], xpwT_d[j, k * P:(k + 1) * P, :])
                for n in range(NT):
                    nc.tensor.matmul(
                        ps_xd[n][0:96, :], xw[:], u16[:, k, n * 512:(n + 1) * 512],
                        start=(k == 0), stop=(k == MF - 1))
            xd16 = prm.tile([96, L], BF16, name="xd16")
            for n in range(NT):
                sl = slice(n * 512, (n + 1) * 512)
                nc.vector.tensor_copy(xd16[0:DR, sl], ps_xd[n][0:DR, :])
                nc.scalar.copy(xd16[64:96, sl], ps_xd[n][64:96, :])
            nc.sync.dma_start(bc_dram[:], xd16[64:96, :])

            # ---- scan chunks ----
            for ch in range(NCH):
                fs = slice(ch * CH, (ch + 1) * CH)
                # dt_proj for this chunk (K=48), + exp/+1/ln -> lnr (= delta)
                ps_dt = [mm.tile([P, 512], F32, name="ps_dt", tag="ps")
                         for _ in range(CH * NT)]
                dw = wbig.tile([DR, CH * P], BF16, name="dw_dt", tag="w")
                nc.sync.dma_start(dw[:], dtwT_d[j, :, ch * CH * P:(ch + 1) * CH * P])
                for f in range(CH):
                    for n in range(NT):
                        nc.tensor.matmul(
                            ps_dt[f * NT + n], dw[:, f * P:(f + 1) * P],
                            xd16[0:DR, n * 512:(n + 1) * 512], start=True, stop=True)
                q16 = chkp.tile([P, CH, L], BF16, name="q16", tag="qr")
                for f in range(CH):
                    for n in range(NT):
                        nc.scalar.activation(
                            q16[:, f, n * 512:(n + 1) * 512], ps_dt[f * NT + n],
                            AF.Exp, bias=dtbn[:, ch * CH + f:ch * CH + f + 1])
                lnr = chk1.tile([P, CH, L], BF16, name="lnr", tag="lnrp")
                nc.scalar.activation(lnr[:].rearrange("p a b -> p (a b)"),
                                     q16[:].rearrange("p a b -> p (a b)"), AF.Ln,
                                     bias=onef[:])
                w16 = chk1.tile([P, CH, L], BF16, name="w16c", tag="wcp")
                nc.vector.tensor_tensor(out=w16[:], in0=lnr[:], in1=u16[:, fs, :],
                                        op=MUL)
                # poison t=0 so scans reset at fold boundaries:
                # exp(-a_i * 30000) == 0
                nc.vector.memset(lnr[:, :, 0:1], 30000.0)

                acc = None
                for i in range(DS):
                    nb_i = cbcp.tile([P, L], BF16, name="nb_bc", tag="nbcb")
                    nc.sync.dma_start(nb_i[:], bc_dram[i:i + 1, :]
                                      .to_broadcast([P, L]))
                    cb_i = cbcp.tile([P, L], BF16, name="cb_bc", tag="nbcb")
                    nc.sync.dma_start(cb_i[:], bc_dram[DS + i:DS + i + 1, :]
                                      .to_broadcast([P, L]))
                    dA = dApool.tile([P, CH, L], BF16, name="dA")
                    nc.scalar.activation(
                        dA[:].rearrange("p a b -> p (a b)"),
                        lnr[:].rearrange("p a b -> p (a b)"),
                        AF.Exp, scale=napos[:, i:i + 1])
                    dB = dBpool.tile([P, CH, L], BF16, name="dB", tag="dBp")
                    nc.vector.tensor_tensor(
                        out=dB[:], in0=w16[:],
                        in1=nb_i[:, None, :].to_broadcast([P, CH, L]), op=MUL)
                    s16 = spool.tile([P, CH, L], BF16, name="s16")
                    nc.vector.tensor_tensor_scan(
                        s16[:].rearrange("p a b -> p (a b)"),
                        dA[:].rearrange("p a b -> p (a b)"),
                        dB[:].rearrange("p a b -> p (a b)"),
                        0.0, MUL, ADD)
                    prod = dBpool.tile([P, CH, L], BF16, name="prod", tag="dBp")
                    nc.vector.tensor_tensor(
                        out=prod[:], in0=s16[:],
                        in1=cb_i[:, None, :].to_broadcast([P, CH, L]), op=MUL)
                    tgt = accp.tile([P, CH, L], BF16, name="acc")
                    if i == 0:
                        for fo in range(CH):
                            nc.vector.scalar_tensor_tensor(
                                out=tgt[:, fo, :], in0=u16[:, ch * CH + fo, :],
                                scalar=dsk[:, ch * CH + fo:ch * CH + fo + 1],
                                in1=prod[:, fo, :], op0=MUL, op1=ADD)
                    else:
                        nc.vector.tensor_tensor(out=tgt[:], in0=acc[:], in1=prod[:], op=ADD)
                    acc = tgt
                # gate: g = y * silu(z) -> u16 chunk (u dead after D_skip init)
                nc.vector.tensor_tensor(out=u16[:, fs, :], in0=acc[:],
                                        in1=zs16[:, fs, :], op=MUL)

            # ---- out_proj (g lives in zs16) ----
            for half in range(2):
                ms = range(half * 3, half * 3 + 3)
                ps_o = {(m, n): mm.tile([P, 512], F32, name="ps_op", tag="ps")
                        for m in ms for n in range(NT)}
                for k in range(MF):
                    ow = wbig.tile([P, D], BF16, name="ow_op", tag="w")
                    nc.sync.dma_start(ow[:], owT_d[j, k * P:(k + 1) * P, :])
                    for m in ms:
                        for n in range(NT):
                            nc.tensor.matmul(
                                ps_o[(m, n)], ow[:, m * P:(m + 1) * P],
                                u16[:, k, n * 512:(n + 1) * 512],
                                start=(k == 0), stop=(k == MF - 1))
                for m in ms:
                    for n in range(NT):
                        o_ev = stg32.tile([P, 512], F32, name="stg_ev", tag="st32")
                        if (m + n) % 2 == 0:
                            nc.vector.tensor_copy(o_ev[:], ps_o[(m, n)])
                        else:
                            nc.scalar.copy(o_ev[:], ps_o[(m, n)])
                        nc.sync.dma_start(
                            cc_in[j][m * P:(m + 1) * P, n * 512:(n + 1) * 512],
                            o_ev[:])

            if os.environ.get("K_SIM"):
                nc.sync.dma_start(cc_out[j][:], cc_in[j][:])
            else:
                nc.gpsimd.collective_compute(
                    kind="AllReduce", op=ADD,
                    replica_groups=[[0, 4], [1, 5], [2, 6], [3, 7]],
                    ins=[cc_in[j][:]], outs=[cc_out[j][:]])
            h_new = pers.tile([P, KF, L], F32, name="h32", tag="h32")
            nc.sync.dma_start(h_new[:], cc_out[j][:].rearrange("(f p) l -> p f l", p=P))
            h32 = h_new

        # ---- final: out = LN(h + res) ----
        vf16 = big.tile([P, KF, L], BF16, name="vf16", tag="bigC")
        nc.vector.tensor_tensor(out=vf16[:], in0=h32[:], in1=res16[:], op=ADD)
        fw = prm.tile([P, KF], F32, name="fw")
        fb = prm.tile([P, KF], F32, name="fb")
        nc.sync.dma_start(fw[:], fw_d[:])
        nc.sync.dma_start(fb[:], fb_d[:])
        ofin = big.tile([P, KF, L], BF16, name="ofin", tag="bigA")
        ln_feature_major(vf16, fw, fb, ofin, 0)
        for f in range(KF):
            o_st = stg32.tile([P, L], F32, name="stg_f3", tag="st32")
            nc.vector.tensor_copy(o_st[:], ofin[:, f, :])
            nc.sync.dma_start(o_d[f * P:(f + 1) * P, :], o_st[:])

    nc.compile()
    return nc


def _fold(x):
    x = np.asarray(x, np.float32)
    nf = x.shape[-1] // P
    return np.ascontiguousarray(x.reshape(nf, P).T)


def _prep_core_inputs(inputs, b, d):
    bf = lambda x: np.ascontiguousarray(np.asarray(x)).astype(ml_dtypes.bfloat16)
    f32 = lambda x: np.ascontiguousarray(np.asarray(x, np.float32))
    inp = {k: np.asarray(v) for k, v in inputs.items()}

    inwT = np.empty((NL, D, 2 * DI), np.float32)
    cwf = np.empty((NL, P, MF, DC), np.float32)
    lnw = np.empty((NL, P, KF), np.float32)
    lnb = np.empty((NL, P, KF), np.float32)
    cb = np.empty((NL, P, MF), np.float32)
    xpwT = np.zeros((NL, DI, 96), np.float32)
    dtwT = np.empty((NL, DR, DI), np.float32)
    dtbn = np.empty((NL, P, MF), np.float32)
    napos = np.empty((NL, P, DS), np.float32)
    dsk = np.empty((NL, P, MF), np.float32)
    owT = np.empty((NL, DI, D), np.float32)
    for j in range(NL):
        iw = np.asarray(inp["in_proj_w"][j, d], np.float32)   # (3072, 768)
        cw = np.asarray(inp["conv_w"][j, d], np.float32)      # (1536, 4)
        inwT[j] = iw.T
        cwf[j] = cw.reshape(MF, P, DC).transpose(1, 0, 2)
        lnw[j] = _fold(inp["norm_w"][j, d])
        lnb[j] = _fold(inp["norm_b"][j, d])
        cb[j] = _fold(inp["conv_b"][j, d])
        xpw_t = np.asarray(inp["x_proj_w"][j, d], np.float32).T
        xpwT[j, :, 0:DR] = xpw_t[:, 0:DR]
        xpwT[j, :, 64:80] = xpw_t[:, DR:DR + DS]
        xpwT[j, :, 80:96] = xpw_t[:, DR + DS:80]
        dtwT[j] = np.asarray(inp["dt_proj_w"][j, d], np.float32).T
        dtbn[j] = _fold(np.asarray(inp["dt_proj_b"][j, d], np.float32))
        a = np.exp(np.asarray(inp["A_log"][j, d], np.float32))  # (1536, 16)
        assert np.allclose(a, a[0:1, :], rtol=1e-5), "A_log not d-constant"
        napos[j] = np.tile(-a[0], (P, 1))
        dsk[j] = _fold(inp["D_skip"][j, d])
        owT[j] = np.asarray(inp["out_proj_w"][j, d], np.float32).T

    sel = np.zeros((P, 2), np.float32)
    sel[:, d] = 1.0
    return {
        "x0T": f32(np.asarray(inp["input_data"][b], np.float32).T),
        "lnw": f32(lnw), "lnb": f32(lnb),
        "inwT": bf(inwT), "cwf": f32(cwf), "cb": f32(cb),
        "xpwT": bf(xpwT), "dtwT": bf(dtwT), "dtbn": f32(dtbn),
        "napos": f32(napos), "dsk": f32(dsk), "owT": bf(owT),
        "fw": f32(_fold(inp["norm_f_w"])), "fb": f32(_fold(inp["norm_f_b"])),
        "sel": sel,
    }


def kernel(**inputs):
    if "nc" not in _CACHE:
        _CACHE["nc"] = _build()
    nc = _CACHE["nc"]
    in_maps = [_prep_core_inputs(inputs, c % 4, c // 4) for c in range(8)]
    try:
        res = run_bass_kernel_spmd(nc, in_maps, core_ids=list(range(8)))
    except Exception:
        import time as _time
        _time.sleep(5)
        res = run_bass_kernel_spmd(nc, in_maps, core_ids=list(range(8)))
    out = np.empty((B, L, D), np.float32)
    for b in range(B):
        out[b] = res.results[b]["o_fm"].T
    return out
